# revision 1
# baseline (speedup 1.0000x reference)
"""Trainium2 Bass kernel for nn_Block_52278341927299 (dense transformer block).

Sharding: H-dim split 8 ways (2 rows of 16 per core -> 1024 contiguous
spatial positions each). MLP + qkv convs fully local; k/v AllGathered
across cores for attention (each core computes attention for its 1024
query positions over all 8192 keys).

Self-contained: hardcodes shapes; only depends on the system toolchain
at /opt/trn_rl_repo.
"""
import sys

if '/opt/trn_rl_repo' not in sys.path:
    sys.path.insert(0, '/opt/trn_rl_repo')

import numpy as np
import ml_dtypes

import concourse.bass as bass
import concourse.bacc as bacc
import concourse.mybir as mybir
import concourse.tile as tile
from concourse.bass_utils import run_bass_kernel_spmd
from concourse.masks import make_identity

F32 = mybir.dt.float32
F32R = mybir.dt.float32r
BF16 = mybir.dt.bfloat16
F16 = mybir.dt.float16
AF = mybir.ActivationFunctionType

R = 8            # cores
C = 512          # channels
CT = 4           # channel tiles of 128
M = 1024         # mlp hidden
MT = 8
HEADS = 4
D = 128          # head dim
Lc = 1024        # local positions per core (2 rows x 512)
L = 8192         # total positions
KT = 64          # key tiles of 128
EPS = 1e-4
SILU_SCALE = 0.596
CLIP = 256.0
ISQ_D = 1.0 / np.sqrt(128.0)   # exp scale (1/sqrt(d))
ISQ2 = 1.0 / np.sqrt(2.0)

_CACHE = {}


def _build_nc(reps=1, bench=0, bench_cc=0):
    """Timing builds (collectives cannot sit under runtime control flow on
    this runtime, so they stay straight-line):

    bench=K: body x1 (with its 2 AllGathers) always executes; K-1 extra
      compute-only bodies (collectives skipped, gathered buffers reused) are
      gated behind If(niter>1). One executable; device time scales with the
      niter input -> clean same-executable reps-delta for the compute part.
    bench_cc=m: body x1 plus m extra chained AllGather pairs appended
      straight-line -> cross-executable delta vs the plain build isolates
      the per-AllGather-pair cost, amortized over m.
    """
    nc = bacc.Bacc(num_devices=R)
    niter_d = None
    if bench:
        niter_d = nc.declare_dram_parameter(
            "niter", [1, 1], mybir.dt.int32, isOutput=False)

    # ---------------- I/O ----------------
    # All big tensors are pre-permuted on the host to partition-major
    # [128, ...] layouts so every DMA is a contiguous per-partition stream
    # (descriptor generation on the sync queue is the phase bottleneck
    # otherwise).
    xs_d = nc.declare_dram_parameter("xs", [128, CT * Lc], F32, isOutput=False)
    pos_d = nc.declare_dram_parameter("pos", [128, CT * Lc], F32, isOutput=False)
    emb_d = nc.declare_dram_parameter("embv", [128, CT], F32, isOutput=False)
    w0t_d = nc.declare_dram_parameter("w0t", [128, CT * M], F32, isOutput=False)
    wdt_d = nc.declare_dram_parameter("wdt", [128, 72 * 128], F32, isOutput=False)
    wembt_d = nc.declare_dram_parameter("wembt", [128, CT * M], F32, isOutput=False)
    w1t_d = nc.declare_dram_parameter("w1t", [128, MT * C], F32, isOutput=False)
    wet_d = nc.declare_dram_parameter("wet", [128, CT * M], F32, isOutput=False)
    wot_d = nc.declare_dram_parameter("wot", [128, CT * M], F32, isOutput=False)
    wvt_d = nc.declare_dram_parameter("wvt", [128, CT * C], F32, isOutput=False)
    onec_d = nc.declare_dram_parameter("onec", [128, 1], F32, isOutput=False)
    oner_d = nc.declare_dram_parameter("oner", [1, 128], F32, isOutput=False)
    halfr_d = nc.declare_dram_parameter("halfr", [1, 128], F32, isOutput=False)
    onecb_d = nc.declare_dram_parameter("onecb", [128, 1], mybir.dt.bfloat16, isOutput=False)
    out_d = nc.declare_dram_parameter("out", [128, CT * Lc], F32, isOutput=True)

    # internal DRAM for the merged collective: slot 0 = k (fp16 [C, Lc]),
    # slot 1 = v (bf16 [Lc, C] bit pattern in an fp16-typed buffer)
    agm_in = nc.dram_tensor("agm_in", [2, C * Lc], F16)
    agm_out = nc.dram_tensor("agm_out", [R, 2, C * Lc], F16,
                             addr_space="Shared")

    with tile.TileContext(nc) as tc:
        with tc.tile_pool(name="persist", bufs=1) as pp:
            # constants
            onec = pp.tile([128, 1], F32R)
            nc.sync.dma_start(onec[:], onec_d[:].bitcast(F32R))
            oner = pp.tile([1, 128], F32R)
            nc.sync.dma_start(oner[:], oner_d[:].bitcast(F32R))
            halfr = pp.tile([1, 128], F32R)
            nc.sync.dma_start(halfr[:], halfr_d[:].bitcast(F32R))
            onecb = pp.tile([128, 1], BF16)
            nc.sync.dma_start(onecb[:], onecb_d[:])
            ident = pp.tile([128, 128], F32)
            make_identity(nc, ident[:])
            zer8 = pp.tile([128, 8], F32)
            nc.any.memset(zer8[:], 0.0)
            # cross-phase tensors
            xm = pp.tile([128, CT, Lc], F32R)      # sqrt(2) * x_mid
            q_sb = pp.tile([128, HEADS, Lc], F16)  # normalized q (pre 1/sqrt(d))
            xout = pp.tile([128, CT, Lc], F32)      # attn out (scaled 1/sqrt2)
            xh = pp.tile([128, CT, Lc], F32)        # xm/2, precomputed during AG
            c_col = pp.tile([128, MT], F32)         # emb modulation, column layout

            env = locals()
            if bench:
                nv = nc.values_load(niter_d[0:1, 0:1], min_val=0,
                                    max_val=1 << 20,
                                    skip_runtime_bounds_check=True)
                _build_body(nc, tc, pp, env)
                with tc.If(nv > 1):
                    for _rep in range(bench - 1):
                        _build_body(nc, tc, pp, env, skip_cc=True)
            elif bench_cc:
                _build_body(nc, tc, pp, env)
                for _rep in range(bench_cc):
                    _emit_cc_pair(nc, env)
            else:
                for _rep in range(reps):
                    _build_body(nc, tc, pp, env)
    nc.compile()
    return nc


def _emit_cc_pair(nc, env):
    nc.gpsimd.collective_compute(
        "AllGather", mybir.AluOpType.bypass,
        replica_groups=[list(range(R))],
        ins=[env["agm_in"][:]], outs=[env["agm_out"][:]])


def _build_body(nc, tc, pp, env, skip_cc=False):
    onec = env["onec"]; oner = env["oner"]; halfr = env["halfr"]
    onecb = env["onecb"]; ident = env["ident"]; zer8 = env["zer8"]
    xm = env["xm"]; q_sb = env["q_sb"]; xout = env["xout"]; c_col = env["c_col"]
    xh = env["xh"]
    xs_d = env["xs_d"]; pos_d = env["pos_d"]; emb_d = env["emb_d"]
    w0t_d = env["w0t_d"]; wdt_d = env["wdt_d"]; wembt_d = env["wembt_d"]
    w1t_d = env["w1t_d"]; wet_d = env["wet_d"]; wot_d = env["wot_d"]
    wvt_d = env["wvt_d"]; out_d = env["out_d"]
    agm_in = env["agm_in"]; agm_out = env["agm_out"]
    if True:
            # ============ Phase 1+2: x-norm + MLP ============
            with tc.tile_pool(name="pA", bufs=1) as pA, \
                 tc.tile_pool(name="pAps", bufs=1, space="PSUM") as pAps, \
                 tc.tile_pool(name="pw", bufs=1) as pw:
                xn = pA.tile([128, CT, Lc], F32R)
                y2 = pA.tile([128, MT, Lc], F32R)

                with tc.tile_pool(name="p1", bufs=1) as p1, \
                     tc.tile_pool(name="p1ps", bufs=1, space="PSUM") as p1ps:
                    xs = p1.tile([128, CT, Lc], F32R)
                    nc.sync.dma_start(
                        xs[:],
                        xs_d[:].rearrange("p (t l) -> p t l", l=Lc).bitcast(F32R))
                    # prefetch res0/depth weights while the norm chain runs
                    w0t_sb = pw.tile([128, CT, M], F32R)
                    nc.sync.dma_start(
                        w0t_sb[:],
                        w0t_d[:].rearrange("p (t co) -> p t co", co=M).bitcast(F32R))
                    wdt_sb = pw.tile([128, 72, 128], F32R)
                    nc.sync.dma_start(
                        wdt_sb[:],
                        wdt_d[:].rearrange("p (s co) -> p s co", co=128).bitcast(F32R))
                    nsx = p1ps.tile([1, Lc], F32)
                    for t in range(CT):
                        sq = p1.tile([128, Lc], BF16, tag="sq", bufs=2)
                        nc.vector.tensor_mul(sq[:], xs[:, t, :], xs[:, t, :])
                        for ch in range(2):
                            nc.tensor.matmul(
                                nsx[0:1, ch * 512:(ch + 1) * 512],
                                onecb[:],
                                sq[:, ch * 512:(ch + 1) * 512],
                                start=(t == 0), stop=(t == CT - 1))
                    sn = p1ps.tile([1, Lc], F32)
                    nc.scalar.activation(sn[:], nsx[:], AF.Sqrt, scale=1.0 / C)
                    nc.vector.tensor_scalar_add(sn[:], sn[:], EPS)
                    inv_x = p1.tile([1, Lc], F32)
                    scr1 = p1.tile([1, Lc], F32)
                    nc.vector.reciprocal_approx_accurate(inv_x[:], sn[:], scr1[:])
                    inv_xr = p1.tile([1, Lc], F32R)
                    nc.vector.tensor_copy(inv_xr[:], inv_x[:])
                    invbc = p1ps.tile([128, Lc], F32)
                    for ch in range(2):
                        nc.tensor.matmul(
                            invbc[:, ch * 512:(ch + 1) * 512],
                            oner[:],
                            inv_xr[0:1, ch * 512:(ch + 1) * 512],
                            start=True, stop=True)
                    for t in range(CT):
                        nc.vector.tensor_mul(
                            xn[:, t, :], xs[:, t, :], invbc[:])

                # ============ Phase 0: emb modulation c ============
                with tc.tile_pool(name="p0", bufs=1) as p0, \
                     tc.tile_pool(name="p0ps", bufs=1, space="PSUM") as p0ps:
                    wembt_sb = p0.tile([128, CT, M], F32)
                    nc.sync.dma_start(
                        wembt_sb[:],
                        wembt_d[:].rearrange("p (t co) -> p t co", co=M))
                    emb_sb = p0.tile([128, CT], F32)
                    nc.sync.dma_start(emb_sb[:], emb_d[:])
                    cps = p0ps.tile([128, MT], F32)
                    for g in range(MT):
                        for t in range(CT):
                            nc.tensor.matmul(
                                cps[:, g:g + 1],
                                wembt_sb[:, t, g * 128:(g + 1) * 128],
                                emb_sb[:, t:t + 1],
                                start=(t == 0), stop=(t == CT - 1))
                    nc.scalar.add(c_col[:], cps[:], 1.0)

                # ---- res0 + depth conv + silu ----
                with tc.tile_pool(name="p2a", bufs=1) as p2a, \
                     tc.tile_pool(name="p2aps", bufs=1, space="PSUM") as p2aps:
                    # prefetch res1 weights during the depth conv
                    w1t_sb = pw.tile([128, MT, C], F32R)
                    nc.sync.dma_start(
                        w1t_sb[:],
                        w1t_d[:].rearrange("p (g co) -> p g co", co=C).bitcast(F32R))
                    for g in range(MT):
                        y0ps = p2aps.tile([128, Lc], F32, tag="y0ps", bufs=3)
                        for ch in range(2):
                            for t in range(CT):
                                nc.tensor.matmul(
                                    y0ps[:, ch * 512:(ch + 1) * 512],
                                    w0t_sb[:, t, g * 128:(g + 1) * 128],
                                    xn[:, t, ch * 512:(ch + 1) * 512],
                                    start=(t == 0), stop=(t == CT - 1))
                        # zero-padded (+4 each side) even/odd-shift copies so all
                        # fp32r tap matmuls are full-width with even offsets
                        y0e = p2a.tile([128, 2, 520], F32R, tag="y0e", bufs=3)
                        y0o = p2a.tile([128, 2, 520], F32R, tag="y0o", bufs=3)
                        nc.vector.tensor_copy(
                            y0e[:, :, 0:4],
                            zer8[:, None, 0:4].broadcast_to([128, 2, 4]))
                        nc.vector.tensor_copy(
                            y0e[:, :, 516:520],
                            zer8[:, None, 0:4].broadcast_to([128, 2, 4]))
                        nc.vector.tensor_copy(
                            y0o[:, :, 0:3],
                            zer8[:, None, 0:3].broadcast_to([128, 2, 3]))
                        nc.vector.tensor_copy(
                            y0o[:, :, 515:520],
                            zer8[:, None, 0:5].broadcast_to([128, 2, 5]))
                        for row in range(2):
                            nc.vector.tensor_copy(
                                y0e[:, row, 4:516],
                                y0ps[:, row * 512:(row + 1) * 512])
                            nc.vector.tensor_copy(
                                y0o[:, row, 3:515],
                                y0ps[:, row * 512:(row + 1) * 512])
                        for row in range(2):
                            y1ps = p2aps.tile([128, 512], F32, tag="y1ps", bufs=2)
                            for tap in range(9):
                                if tap % 2 == 0:
                                    rhs = y0e[:, row, tap:tap + 512]
                                else:
                                    rhs = y0o[:, row, tap - 1:tap - 1 + 512]
                                nc.tensor.matmul(
                                    y1ps[:],
                                    wdt_sb[:, g * 9 + tap, :],
                                    rhs,
                                    start=(tap == 0), stop=(tap == 8))
                            nc.scalar.activation(
                                y2[:, g, row * 512:(row + 1) * 512],
                                y1ps[:],
                                AF.Silu,
                                scale=c_col[:, g:g + 1])

                # ---- res1 + x_mid ----
                with tc.tile_pool(name="p2b", bufs=1) as p2b, \
                     tc.tile_pool(name="p2bps", bufs=1, space="PSUM") as p2bps:
                    for ch in range(2):
                        for mo in range(CT):
                            y3ps = p2bps.tile([128, 512], F32, tag="y3ps", bufs=4)
                            for g in range(MT):
                                nc.tensor.matmul(
                                    y3ps[:],
                                    w1t_sb[:, g, mo * 128:(mo + 1) * 128],
                                    y2[:, g, ch * 512:(ch + 1) * 512],
                                    start=(g == 0), stop=(g == MT - 1))
                            nc.vector.tensor_add(
                                xm[:, mo, ch * 512:(ch + 1) * 512],
                                xn[:, mo, ch * 512:(ch + 1) * 512],
                                y3ps[:])

            # ============ Phase 3: qkv (transposed) ============
            with tc.tile_pool(name="p3", bufs=1) as p3, \
                 tc.tile_pool(name="p3ps", bufs=1, space="PSUM") as p3ps:
                pos = p3.tile([128, CT, Lc], F32)
                nc.sync.dma_start(
                    pos[:], pos_d[:].rearrange("p (t l) -> p t l", l=Lc))
                wet_sb = p3.tile([128, CT, M], F32R)
                nc.sync.dma_start(
                    wet_sb[:],
                    wet_d[:].rearrange("p (t co) -> p t co", co=M).bitcast(F32R))
                wot_sb = p3.tile([128, CT, M], F32R)
                nc.sync.dma_start(
                    wot_sb[:],
                    wot_d[:].rearrange("p (t co) -> p t co", co=M).bitcast(F32R))
                wvt_sb = p3.tile([128, CT, C], F32R)
                nc.sync.dma_start(
                    wvt_sb[:],
                    wvt_d[:].rearrange("p (t co) -> p t co", co=C).bitcast(F32R))
                xpos = p3.tile([128, CT, Lc], F32R)
                nc.vector.tensor_mul(xpos[:], xm[:], pos[:])

                qkn = p3.tile([128, 8, Lc], F32)    # [l%128, ltile, co] normalized qk^T
                vn = p3.tile([128, 8, C], BF16)      # normalized v^T
                stats = p3.tile([128, 8, 12], F32)

                for lt in range(8):
                    qt0 = p3ps.tile([128, 512], F32, tag="qkvps", bufs=8)
                    qt1 = p3ps.tile([128, 512], F32, tag="qkvps", bufs=8)
                    vtp = p3ps.tile([128, 512], F32, tag="qkvps", bufs=8)
                    for t in range(CT):
                        nc.tensor.matmul(
                            qt0[:], xm[:, t, lt * 128:(lt + 1) * 128],
                            wet_sb[:, t, 0:512],
                            start=(t == 0), stop=False)
                    for t in range(CT):
                        nc.tensor.matmul(
                            qt0[:], xpos[:, t, lt * 128:(lt + 1) * 128],
                            wot_sb[:, t, 0:512],
                            start=False, stop=(t == CT - 1))
                    for t in range(CT):
                        nc.tensor.matmul(
                            qt1[:], xm[:, t, lt * 128:(lt + 1) * 128],
                            wet_sb[:, t, 512:1024],
                            start=(t == 0), stop=False)
                    for t in range(CT):
                        nc.tensor.matmul(
                            qt1[:], xpos[:, t, lt * 128:(lt + 1) * 128],
                            wot_sb[:, t, 512:1024],
                            start=False, stop=(t == CT - 1))
                    for t in range(CT):
                        nc.tensor.matmul(
                            vtp[:], xm[:, t, lt * 128:(lt + 1) * 128],
                            wvt_sb[:, t, :],
                            start=(t == 0), stop=(t == CT - 1))
                    sq3 = p3.tile([128, 1536], F32, tag="sq3", bufs=2)
                    nc.scalar.square(sq3[:, 0:512], qt0[:])
                    nc.scalar.square(sq3[:, 512:1024], qt1[:])
                    nc.scalar.square(sq3[:, 1024:1536], vtp[:])
                    nc.vector.tensor_reduce(
                        stats[:, lt, :],
                        sq3[:].rearrange("p (s d) -> p s d", d=128),
                        axis=mybir.AxisListType.X,
                        op=mybir.AluOpType.add)
                    inv_t = p3.tile([128, 12], F32, tag="inv12", bufs=2)
                    scr_t = p3.tile([128, 12], F32, tag="scr12", bufs=2)
                    tmp_t = p3.tile([128, 12], F32, tag="tmp12", bufs=2)
                    nc.scalar.activation(
                        tmp_t[:], stats[:, lt, :], AF.Sqrt, scale=1.0 / 128.0)
                    nc.vector.tensor_scalar_add(tmp_t[:], tmp_t[:], EPS)
                    nc.vector.reciprocal_approx_accurate(scr_t[:], tmp_t[:], inv_t[:])
                    # scr_t now holds 1/(eps+sqrt(ns/128)); apply
                    nc.vector.tensor_mul(
                        qkn[:, lt, 0:512],
                        qt0[:].rearrange("p (s d) -> p s d", d=128),
                        scr_t[:, 0:4, None].broadcast_to([128, 4, 128]))
                    nc.vector.tensor_mul(
                        qkn[:, lt, 512:1024],
                        qt1[:].rearrange("p (s d) -> p s d", d=128),
                        scr_t[:, 4:8, None].broadcast_to([128, 4, 128]))
                    nc.vector.tensor_mul(
                        vn[:, lt, :],
                        vtp[:].rearrange("p (s d) -> p s d", d=128),
                        scr_t[:, 8:12, None].broadcast_to([128, 4, 128]))

                # k transposes (fp16 payload) + v ship, then one merged AG;
                # q transposes overlap the collective
                agk_view = agm_in[0, :].rearrange("(c l) -> c l", c=C)
                for h in range(HEADS):
                    for bank in range(2):
                        tp = p3ps.tile([128, 512], F32, tag="qkvps", bufs=8)
                        for i in range(4):
                            lt = bank * 4 + i
                            nc.tensor.transpose(
                                tp[:, i * 128:(i + 1) * 128],
                                qkn[:, lt, (2 * h + 1) * 128:(2 * h + 2) * 128],
                                ident[:])
                        kst = p3.tile([128, 512], F16, tag="kst", bufs=2)
                        nc.vector.tensor_copy(kst[:], tp[:])
                        nc.sync.dma_start(
                            agk_view[h * 128:(h + 1) * 128,
                                     bank * 512:(bank + 1) * 512],
                            kst[:])
                nc.sync.dma_start(
                    agm_in[1, :].bitcast(BF16)
                    .rearrange("(lt p d) -> p lt d", p=128, d=C), vn[:])
                nc.vector.tensor_scalar_mul(xh[:], xm[:], 0.5)
                if not skip_cc:
                    nc.gpsimd.collective_compute(
                        "AllGather", mybir.AluOpType.bypass,
                        replica_groups=[list(range(R))],
                        ins=[agm_in[:]], outs=[agm_out[:]])
                for h in range(HEADS):
                    for bank in range(2):
                        tp = p3ps.tile([128, 512], F32, tag="qkvps", bufs=8)
                        for i in range(4):
                            lt = bank * 4 + i
                            nc.tensor.transpose(
                                tp[:, i * 128:(i + 1) * 128],
                                qkn[:, lt, (2 * h) * 128:(2 * h + 1) * 128],
                                ident[:])
                        nc.vector.tensor_copy(
                            q_sb[:, h, bank * 512:(bank + 1) * 512], tp[:])

            # ============ Phase 4: attention ============
            with tc.tile_pool(name="p4", bufs=1) as p4, \
                 tc.tile_pool(name="p4ps", bufs=1, space="PSUM") as p4ps:
                # one full-d v tile for all heads: contiguous gather, head
                # slices taken directly by the PV matmuls
                def load_ksb(hh):
                    t_ = p4.tile([128, L], F16, tag="ksb", bufs=2)
                    for r_ in range(R):
                        nc.sync.dma_start(
                            t_[:, r_ * Lc:(r_ + 1) * Lc],
                            agm_out[r_, 0, :]
                            .rearrange("(c l) -> c l", c=C)
                            [hh * 128:(hh + 1) * 128, :])
                    return t_

                ksb_next = load_ksb(0)
                vsb = p4.tile([128, KT, C], BF16)
                for r_ in range(R):
                    nc.sync.dma_start(
                        vsb[:, r_ * 8:(r_ + 1) * 8, :],
                        agm_out[r_, 1, :].bitcast(BF16)
                        .rearrange("(t p d) -> p t d", p=128, d=C))
                for h in range(HEADS):
                    ksb = ksb_next
                    # prefetch next head's k before this head's output write
                    # enters the (in-order) DMA queue
                    if h + 1 < HEADS:
                        ksb_next = load_ksb(h + 1)
                    for cch in range(2):
                        q_c = q_sb[:, h, cch * 512:(cch + 1) * 512]
                        pvps = p4ps.tile([128, 512], F32, tag="pvps", bufs=2)
                        smps = p4ps.tile([1, 512], F32, tag="smps", bufs=1)
                        g32s = []
                        e_prev = None
                        for grp in range(KT // 2):
                            sps = p4ps.tile([128, 1024], F32, tag="sps", bufs=2)
                            for j in range(2):
                                t = grp * 2 + j
                                nc.tensor.matmul(
                                    sps[:, j * 512:(j + 1) * 512],
                                    ksb[:, t * 128:(t + 1) * 128],
                                    q_c,
                                    start=True, stop=True)
                            e = p4.tile([128, 1024], BF16, tag="e", bufs=4)
                            nc.scalar.activation(e[:], sps[:], AF.Exp, scale=ISQ_D)
                            # PV for the PREVIOUS group: its exp has had a full
                            # score-round to finish, so PE never stalls on ACT
                            if e_prev is not None:
                                pg = grp - 1
                                for j in range(2):
                                    t = pg * 2 + j
                                    nc.tensor.matmul(
                                        pvps[:],
                                        vsb[:, t, h * 128:(h + 1) * 128],
                                        e_prev[:, j * 512:(j + 1) * 512],
                                        start=(t == 0), stop=False)
                            e_prev = e
                            g32 = p4.tile([128, 512], BF16, tag="g32", bufs=6)
                            nc.vector.tensor_add(
                                g32[:], e[:, 0:512], e[:, 512:1024])
                            g32s.append(g32)
                            if grp % 4 == 3:
                                g16a = p4.tile([128, 512], BF16, tag="g16",
                                               bufs=4)
                                nc.vector.tensor_add(
                                    g16a[:], g32s[0][:], g32s[1][:])
                                g16b = p4.tile([128, 512], BF16, tag="g16",
                                               bufs=4)
                                nc.vector.tensor_add(
                                    g16b[:], g32s[2][:], g32s[3][:])
                                g8 = p4.tile([128, 512], BF16, tag="g8",
                                             bufs=2)
                                nc.vector.tensor_add(g8[:], g16a[:], g16b[:])
                                nc.tensor.matmul(
                                    smps[0:1, :],
                                    onecb[:],
                                    g8[:],
                                    start=(grp == 3),
                                    stop=(grp == KT // 2 - 1))
                                g32s = []
                        pg = KT // 2 - 1
                        for j in range(2):
                            t = pg * 2 + j
                            nc.tensor.matmul(
                                pvps[:],
                                vsb[:, t, h * 128:(h + 1) * 128],
                                e_prev[:, j * 512:(j + 1) * 512],
                                start=False, stop=(t == KT - 1))
                        inv_s = p4.tile([1, 512], F32, tag="invs", bufs=2)
                        scr_s = p4.tile([1, 512], F32, tag="scrs", bufs=2)
                        nc.vector.reciprocal_approx_accurate(
                            inv_s[:], smps[:], scr_s[:])
                        inv_sr = p4.tile([1, 512], F32R, tag="invsr", bufs=2)
                        nc.vector.tensor_copy(inv_sr[:], inv_s[:])
                        repl = p4ps.tile([128, 512], F32, tag="repl", bufs=1)
                        nc.tensor.matmul(
                            repl[:], halfr[:], inv_sr[0:1, :],
                            start=True, stop=True)
                        pv_sb = p4.tile([128, 512], F32, tag="pvsb", bufs=2)
                        nc.vector.tensor_copy(pv_sb[:], pvps[:])
                        nc.vector.tensor_mul(
                            xout[:, h, cch * 512:(cch + 1) * 512],
                            pv_sb[:],
                            repl[:])

                    # per-head final combine: out = clip(xm/2 + attn, +-256)
                    nc.vector.tensor_add(
                        xh[:, h, :], xh[:, h, :], xout[:, h, :])
                    nc.vector.tensor_scalar(
                        xh[:, h, :], xh[:, h, :], CLIP, -CLIP,
                        op0=mybir.AluOpType.min, op1=mybir.AluOpType.max)
                    nc.sync.dma_start(
                        out_d[:].rearrange("p (t l) -> p t l", l=Lc)[:, h, :],
                        xh[:, h, :])



def _host_prep(x, emb, pos_emb, emb_gain, w_res0, w_depth, w_emb, w_res1,
               w_qk, w_v):
    """Build shared weight arrays + per-core input shards."""
    f = np.float32
    w_res0 = np.asarray(w_res0, f).reshape(M, C)
    w_depth = np.asarray(w_depth, f).reshape(M, 128, 9)
    w_emb = np.asarray(w_emb, f).reshape(M, C)
    w_res1 = np.asarray(w_res1, f).reshape(C, M)
    w_qk = np.asarray(w_qk, f).reshape(2 * C, 2 * C)
    w_v = np.asarray(w_v, f).reshape(C, C)
    emb_gain = np.float32(emb_gain)

    w0t = np.ascontiguousarray((w_res0 * (1.0 / np.sqrt(C))).T)     # [C, M]
    wd = w_depth * (1.0 / np.sqrt(128 * 9))
    # wdt[p, (g*9+t)*128 + co] = wd[g*128+co, p, t]
    wdt = np.empty((128, 72 * 128), f)
    for g in range(MT):
        blk = wd[g * 128:(g + 1) * 128]          # [co=128, ci=128, tap=9]
        wdt[:, g * 9 * 128:(g + 1) * 9 * 128] = (
            blk.transpose(1, 2, 0).reshape(128, 9 * 128))
    wembt = np.ascontiguousarray((w_emb * (emb_gain / np.sqrt(C))).T)  # [C, M]
    w1t = np.ascontiguousarray(
        (w_res1 * (1.0 / (SILU_SCALE * np.sqrt(M)))).T)             # [M, C]

    # qk permutation: new row n = (2h+s)*128+dd  <- old row h*256 + dd*2 + s
    perm = np.empty(2 * C, np.int64)
    for h in range(HEADS):
        for s in range(2):
            for dd in range(128):
                perm[(2 * h + s) * 128 + dd] = h * 256 + dd * 2 + s
    wqk_p = w_qk[perm] * (1.0 / np.sqrt(2 * C))
    wet = np.ascontiguousarray((wqk_p[:, 0::2] * ISQ2).T)           # [C, M]
    wot = np.ascontiguousarray(wqk_p[:, 1::2].T)                    # [C, M]
    wvt = np.ascontiguousarray((w_v * (1.0 / np.sqrt(C) * ISQ2)).T)  # [C, C]

    x = np.asarray(x, f).reshape(C, 16, 512)
    pos = np.asarray(pos_emb, f).reshape(C, 16, 512) * ISQ2
    embv = np.ascontiguousarray(np.asarray(emb, f).reshape(C))

    def pmajor(a, nt):
        # [nt*128, F] -> [128, nt*F]: partition-major so the DMA streams
        # contiguously per partition
        fdim = a.shape[1]
        return np.ascontiguousarray(
            a.reshape(nt, 128, fdim).transpose(1, 0, 2).reshape(128, nt * fdim))

    shared = dict(
        embv=pmajor(embv.reshape(C, 1), CT),
        w0t=pmajor(w0t, CT), wdt=wdt, wembt=pmajor(wembt, CT),
        w1t=pmajor(w1t, MT), wet=pmajor(wet, CT), wot=pmajor(wot, CT),
        wvt=pmajor(wvt, CT),
        onec=np.ones((128, 1), f), oner=np.ones((1, 128), f),
        halfr=np.full((1, 128), ISQ2, f),
        onecb=np.ones((128, 1), ml_dtypes.bfloat16))
    in_maps = []
    for r in range(R):
        m = dict(shared)
        m["xs"] = pmajor(np.ascontiguousarray(
            x[:, 2 * r:2 * r + 2, :].reshape(C, Lc)), CT)
        m["pos"] = pmajor(np.ascontiguousarray(
            pos[:, 2 * r:2 * r + 2, :].reshape(C, Lc)), CT)
        in_maps.append(m)
    return in_maps


def kernel(**inputs):
    if "nc" not in _CACHE:
        _CACHE["nc"] = _build_nc()
    nc = _CACHE["nc"]
    in_maps = _host_prep(
        inputs["x"], inputs["emb"], inputs["pos_emb"], inputs["emb_gain"],
        inputs["w_res0"], inputs["w_depth"], inputs["w_emb"],
        inputs["w_res1"], inputs["w_qk"], inputs["w_v"])
    res = run_bass_kernel_spmd(nc, in_maps, list(range(R)))
    out = np.empty((1, C, 16, 512), np.float32)
    for r in range(R):
        o = res.results[r]["out"].reshape(128, CT, Lc).transpose(1, 0, 2)
        out[0, :, 2 * r:2 * r + 2, :] = o.reshape(C, 2, 512)
    return out



# revision 10
# speedup vs baseline: 1.0551x; 1.0551x over previous
"""Trainium2 Bass kernel for nn_Block_52278341927299 (dense transformer block).

Sharding: H-dim split 8 ways (2 rows of 16 per core -> 1024 contiguous
spatial positions each). MLP + qkv convs fully local; k/v AllGathered
across cores for attention (each core computes attention for its 1024
query positions over all 8192 keys).

Self-contained: hardcodes shapes; only depends on the system toolchain
at /opt/trn_rl_repo.
"""
import sys

if '/opt/trn_rl_repo' not in sys.path:
    sys.path.insert(0, '/opt/trn_rl_repo')

import numpy as np
import ml_dtypes

import concourse.bass as bass
import concourse.bacc as bacc
import concourse.mybir as mybir
import concourse.tile as tile
from concourse.bass_utils import run_bass_kernel_spmd
from concourse.masks import make_identity

F32 = mybir.dt.float32
F32R = mybir.dt.float32r
BF16 = mybir.dt.bfloat16
F16 = mybir.dt.float16
F8 = mybir.dt.float8e4
U8 = mybir.dt.uint8
AF = mybir.ActivationFunctionType
DR = mybir.MatmulPerfMode.DoubleRow

R = 8            # cores
C = 512          # channels
CT = 4           # channel tiles of 128
M = 1024         # mlp hidden
MT = 8
HEADS = 4
D = 128          # head dim
Lc = 1024        # local positions per core (2 rows x 512)
L = 8192         # total positions
KT = 64          # key tiles of 128
EPS = 1e-4
SILU_SCALE = 0.596
CLIP = 256.0
ISQ_D = 1.0 / np.sqrt(128.0)   # exp scale (1/sqrt(d))
ISQ2 = 1.0 / np.sqrt(2.0)
SHIFT = 6.0                    # exp(s - SHIFT) keeps e <= e^5.31 = 202 < 240 (fp8e4 max)
SCH_A = 8.0 / np.log(2.0)      # Schraudolph fp8e4: bits = rint(y*A + 56 + C)
SCH_C = -0.375
XSPLIT = 640                   # exp columns on ACT; rest on DVE (Schraudolph)

_CACHE = {}


def _build_nc(reps=1, bench=0, bench_cc=0):
    """Timing builds (collectives cannot sit under runtime control flow on
    this runtime, so they stay straight-line):

    bench=K: body x1 (with its 2 AllGathers) always executes; K-1 extra
      compute-only bodies (collectives skipped, gathered buffers reused) are
      gated behind If(niter>1). One executable; device time scales with the
      niter input -> clean same-executable reps-delta for the compute part.
    bench_cc=m: body x1 plus m extra chained AllGather pairs appended
      straight-line -> cross-executable delta vs the plain build isolates
      the per-AllGather-pair cost, amortized over m.
    """
    nc = bacc.Bacc(num_devices=R)
    niter_d = None
    if bench:
        niter_d = nc.declare_dram_parameter(
            "niter", [1, 1], mybir.dt.int32, isOutput=False)

    # ---------------- I/O ----------------
    # All big tensors are pre-permuted on the host to partition-major
    # [128, ...] layouts so every DMA is a contiguous per-partition stream
    # (descriptor generation on the sync queue is the phase bottleneck
    # otherwise).
    xs_d = nc.declare_dram_parameter("xs", [128, CT * Lc], F32, isOutput=False)
    pos_d = nc.declare_dram_parameter("pos", [128, CT * Lc], F32, isOutput=False)
    emb_d = nc.declare_dram_parameter("embv", [128, CT], F32, isOutput=False)
    w0t_d = nc.declare_dram_parameter("w0t", [128, CT * M], F32, isOutput=False)
    wdt_d = nc.declare_dram_parameter("wdt", [128, 72 * 128], F32, isOutput=False)
    wembt_d = nc.declare_dram_parameter("wembt", [128, CT * M], F32, isOutput=False)
    w1t_d = nc.declare_dram_parameter("w1t", [128, MT * C], F32, isOutput=False)
    wet_d = nc.declare_dram_parameter("wet", [128, CT * M], F32, isOutput=False)
    wot_d = nc.declare_dram_parameter("wot", [128, CT * M], F32, isOutput=False)
    wvt_d = nc.declare_dram_parameter("wvt", [128, CT * C], F32, isOutput=False)
    onec_d = nc.declare_dram_parameter("onec", [128, 1], F32, isOutput=False)
    oner_d = nc.declare_dram_parameter("oner", [1, 128], F32, isOutput=False)
    halfr_d = nc.declare_dram_parameter("halfr", [1, 128], F32, isOutput=False)
    onecb_d = nc.declare_dram_parameter("onecb", [128, 1], mybir.dt.bfloat16, isOutput=False)
    out_d = nc.declare_dram_parameter("out", [128, CT * Lc], F32, isOutput=True)

    # internal DRAM for the merged collective (fp16-typed byte carrier):
    # elements [0, C*Lc)            = k fp16 [C, Lc]
    # elements [C*Lc, 3*C*Lc/2)    = v fp8 [lt, p, d] bit pattern
    AGK = C * Lc
    AGV = C * Lc // 2
    agm_in = nc.dram_tensor("agm_in", [1, AGK + AGV], F16)
    agm_out = nc.dram_tensor("agm_out", [R, AGK + AGV], F16,
                             addr_space="Shared")

    with tile.TileContext(nc) as tc:
        with tc.tile_pool(name="persist", bufs=1) as pp:
            # constants
            onec = pp.tile([128, 1], F32R)
            nc.sync.dma_start(onec[:], onec_d[:].bitcast(F32R))
            oner = pp.tile([1, 128], F32R)
            nc.sync.dma_start(oner[:], oner_d[:].bitcast(F32R))
            halfr = pp.tile([1, 128], F32R)
            nc.sync.dma_start(halfr[:], halfr_d[:].bitcast(F32R))
            onecb = pp.tile([128, 1], BF16)
            nc.sync.dma_start(onecb[:], onecb_d[:])
            identh = pp.tile([128, 128], F16)
            make_identity(nc, identh[:])
            zer8 = pp.tile([128, 8], F32)
            nc.any.memset(zer8[:], 0.0)
            ones8 = pp.tile([128, 2, 16], F8)
            nc.any.memset(ones8[:], 1.0)
            nshift = pp.tile([128, 1], F32)
            nc.any.memset(nshift[:], -SHIFT)
            # cross-phase tensors
            xm = pp.tile([128, CT, Lc], F32R)      # sqrt(2) * x_mid
            q_sb = pp.tile([128, HEADS, Lc], F16)  # normalized q (pre 1/sqrt(d))
            xout = pp.tile([128, CT, Lc], F32)      # attn out (scaled 1/sqrt2)
            xh = pp.tile([128, CT, Lc], F32)        # xm/2, precomputed during AG
            c_col = pp.tile([128, MT], F32)         # emb modulation, column layout

            env = locals()
            if bench:
                nv = nc.values_load(niter_d[0:1, 0:1], min_val=0,
                                    max_val=1 << 20,
                                    skip_runtime_bounds_check=True)
                _build_body(nc, tc, pp, env)
                with tc.If(nv > 1):
                    for _rep in range(bench - 1):
                        _build_body(nc, tc, pp, env, skip_cc=True)
            elif bench_cc:
                _build_body(nc, tc, pp, env)
                for _rep in range(bench_cc):
                    _emit_cc_pair(nc, env)
            else:
                for _rep in range(reps):
                    _build_body(nc, tc, pp, env)
    nc.compile()
    return nc


def _emit_cc_pair(nc, env):
    nc.gpsimd.collective_compute(
        "AllGather", mybir.AluOpType.bypass,
        replica_groups=[list(range(R))],
        ins=[env["agm_in"][:]], outs=[env["agm_out"][:]])


def _build_body(nc, tc, pp, env, skip_cc=False):
    onec = env["onec"]; oner = env["oner"]; halfr = env["halfr"]
    onecb = env["onecb"]; identh = env["identh"]; zer8 = env["zer8"]
    ones8 = env["ones8"]; nshift = env["nshift"]
    xm = env["xm"]; q_sb = env["q_sb"]; xout = env["xout"]; c_col = env["c_col"]
    xh = env["xh"]
    AGK = C * Lc
    xs_d = env["xs_d"]; pos_d = env["pos_d"]; emb_d = env["emb_d"]
    w0t_d = env["w0t_d"]; wdt_d = env["wdt_d"]; wembt_d = env["wembt_d"]
    w1t_d = env["w1t_d"]; wet_d = env["wet_d"]; wot_d = env["wot_d"]
    wvt_d = env["wvt_d"]; out_d = env["out_d"]
    agm_in = env["agm_in"]; agm_out = env["agm_out"]
    if True:
            # ============ Phase 1+2: x-norm + MLP ============
            with tc.tile_pool(name="pA", bufs=1) as pA, \
                 tc.tile_pool(name="pAps", bufs=1, space="PSUM") as pAps, \
                 tc.tile_pool(name="pw", bufs=1) as pw:
                xn = pA.tile([128, CT, Lc], F32R)
                y2 = pA.tile([128, MT, Lc], F32R)

                with tc.tile_pool(name="p1", bufs=1) as p1, \
                     tc.tile_pool(name="p1ps", bufs=1, space="PSUM") as p1ps:
                    xs = p1.tile([128, CT, Lc], F32R)
                    nc.sync.dma_start(
                        xs[:],
                        xs_d[:].rearrange("p (t l) -> p t l", l=Lc).bitcast(F32R))
                    # prefetch res0/depth weights while the norm chain runs
                    w0t_sb = pw.tile([128, CT, M], F32R)
                    nc.sync.dma_start(
                        w0t_sb[:],
                        w0t_d[:].rearrange("p (t co) -> p t co", co=M).bitcast(F32R))
                    wdt_sb = pw.tile([128, 72, 128], F32R)
                    nc.sync.dma_start(
                        wdt_sb[:],
                        wdt_d[:].rearrange("p (s co) -> p s co", co=128).bitcast(F32R))
                    nsx = p1ps.tile([1, Lc], F32)
                    for t in range(CT):
                        sq = p1.tile([128, Lc], BF16, tag="sq", bufs=2)
                        nc.vector.tensor_mul(sq[:], xs[:, t, :], xs[:, t, :])
                        for ch in range(2):
                            nc.tensor.matmul(
                                nsx[0:1, ch * 512:(ch + 1) * 512],
                                onecb[:],
                                sq[:, ch * 512:(ch + 1) * 512],
                                start=(t == 0), stop=(t == CT - 1))
                    sn = p1ps.tile([1, Lc], F32)
                    nc.scalar.activation(sn[:], nsx[:], AF.Sqrt, scale=1.0 / C)
                    nc.vector.tensor_scalar_add(sn[:], sn[:], EPS)
                    inv_x = p1.tile([1, Lc], F32)
                    scr1 = p1.tile([1, Lc], F32)
                    nc.vector.reciprocal_approx_accurate(inv_x[:], sn[:], scr1[:])
                    inv_xr = p1.tile([1, Lc], F32R)
                    nc.vector.tensor_copy(inv_xr[:], inv_x[:])
                    invbc = p1ps.tile([128, Lc], F32)
                    for ch in range(2):
                        nc.tensor.matmul(
                            invbc[:, ch * 512:(ch + 1) * 512],
                            oner[:],
                            inv_xr[0:1, ch * 512:(ch + 1) * 512],
                            start=True, stop=True)
                    for t in range(CT):
                        nc.vector.tensor_mul(
                            xn[:, t, :], xs[:, t, :], invbc[:])

                # ============ Phase 0: emb modulation c ============
                with tc.tile_pool(name="p0", bufs=1) as p0, \
                     tc.tile_pool(name="p0ps", bufs=1, space="PSUM") as p0ps:
                    wembt_sb = p0.tile([128, CT, M], F32)
                    nc.sync.dma_start(
                        wembt_sb[:],
                        wembt_d[:].rearrange("p (t co) -> p t co", co=M))
                    emb_sb = p0.tile([128, CT], F32)
                    nc.sync.dma_start(emb_sb[:], emb_d[:])
                    cps = p0ps.tile([128, MT], F32)
                    for g in range(MT):
                        for t in range(CT):
                            nc.tensor.matmul(
                                cps[:, g:g + 1],
                                wembt_sb[:, t, g * 128:(g + 1) * 128],
                                emb_sb[:, t:t + 1],
                                start=(t == 0), stop=(t == CT - 1))
                    nc.scalar.add(c_col[:], cps[:], 1.0)

                # ---- res0 + depth conv + silu ----
                with tc.tile_pool(name="p2a", bufs=1) as p2a, \
                     tc.tile_pool(name="p2aps", bufs=1, space="PSUM") as p2aps:
                    # prefetch res1 weights during the depth conv
                    w1t_sb = pw.tile([128, MT, C], F32R)
                    nc.sync.dma_start(
                        w1t_sb[:],
                        w1t_d[:].rearrange("p (g co) -> p g co", co=C).bitcast(F32R))
                    for g in range(MT):
                        y0ps = p2aps.tile([128, Lc], F32, tag="y0ps", bufs=3)
                        for ch in range(2):
                            for t in range(CT):
                                nc.tensor.matmul(
                                    y0ps[:, ch * 512:(ch + 1) * 512],
                                    w0t_sb[:, t, g * 128:(g + 1) * 128],
                                    xn[:, t, ch * 512:(ch + 1) * 512],
                                    start=(t == 0), stop=(t == CT - 1))
                        # zero-padded (+4 each side) even/odd-shift copies so all
                        # fp32r tap matmuls are full-width with even offsets
                        y0e = p2a.tile([128, 2, 520], F32R, tag="y0e", bufs=3)
                        y0o = p2a.tile([128, 2, 520], F32R, tag="y0o", bufs=3)
                        nc.vector.tensor_copy(
                            y0e[:, :, 0:4],
                            zer8[:, None, 0:4].broadcast_to([128, 2, 4]))
                        nc.vector.tensor_copy(
                            y0e[:, :, 516:520],
                            zer8[:, None, 0:4].broadcast_to([128, 2, 4]))
                        nc.vector.tensor_copy(
                            y0o[:, :, 0:3],
                            zer8[:, None, 0:3].broadcast_to([128, 2, 3]))
                        nc.vector.tensor_copy(
                            y0o[:, :, 515:520],
                            zer8[:, None, 0:5].broadcast_to([128, 2, 5]))
                        for row in range(2):
                            nc.vector.tensor_copy(
                                y0e[:, row, 4:516],
                                y0ps[:, row * 512:(row + 1) * 512])
                            nc.vector.tensor_copy(
                                y0o[:, row, 3:515],
                                y0ps[:, row * 512:(row + 1) * 512])
                        for row in range(2):
                            y1ps = p2aps.tile([128, 512], F32, tag="y1ps", bufs=2)
                            for tap in range(9):
                                if tap % 2 == 0:
                                    rhs = y0e[:, row, tap:tap + 512]
                                else:
                                    rhs = y0o[:, row, tap - 1:tap - 1 + 512]
                                nc.tensor.matmul(
                                    y1ps[:],
                                    wdt_sb[:, g * 9 + tap, :],
                                    rhs,
                                    start=(tap == 0), stop=(tap == 8))
                            nc.scalar.activation(
                                y2[:, g, row * 512:(row + 1) * 512],
                                y1ps[:],
                                AF.Silu,
                                scale=c_col[:, g:g + 1])

                # ---- res1 + x_mid ----
                with tc.tile_pool(name="p2b", bufs=1) as p2b, \
                     tc.tile_pool(name="p2bps", bufs=1, space="PSUM") as p2bps:
                    for ch in range(2):
                        for mo in range(CT):
                            y3ps = p2bps.tile([128, 512], F32, tag="y3ps", bufs=4)
                            for g in range(MT):
                                nc.tensor.matmul(
                                    y3ps[:],
                                    w1t_sb[:, g, mo * 128:(mo + 1) * 128],
                                    y2[:, g, ch * 512:(ch + 1) * 512],
                                    start=(g == 0), stop=(g == MT - 1))
                            nc.vector.tensor_add(
                                xm[:, mo, ch * 512:(ch + 1) * 512],
                                xn[:, mo, ch * 512:(ch + 1) * 512],
                                y3ps[:])

            # ============ Phase 3: qkv (transposed) ============
            with tc.tile_pool(name="p3", bufs=1) as p3, \
                 tc.tile_pool(name="p3ps", bufs=1, space="PSUM") as p3ps:
                pos = p3.tile([128, CT, Lc], F32)
                nc.sync.dma_start(
                    pos[:], pos_d[:].rearrange("p (t l) -> p t l", l=Lc))
                wet_sb = p3.tile([128, CT, M], F32R)
                nc.sync.dma_start(
                    wet_sb[:],
                    wet_d[:].rearrange("p (t co) -> p t co", co=M).bitcast(F32R))
                wot_sb = p3.tile([128, CT, M], F32R)
                nc.sync.dma_start(
                    wot_sb[:],
                    wot_d[:].rearrange("p (t co) -> p t co", co=M).bitcast(F32R))
                wvt_sb = p3.tile([128, CT, C], F32R)
                nc.sync.dma_start(
                    wvt_sb[:],
                    wvt_d[:].rearrange("p (t co) -> p t co", co=C).bitcast(F32R))
                xpos = p3.tile([128, CT, Lc], F32R)
                nc.vector.tensor_mul(xpos[:], xm[:], pos[:])

                qkn = p3.tile([128, 8, Lc], F16)    # [l%128, ltile, co] normalized qk^T
                vn = p3.tile([128, 8, C], F8)        # normalized v^T
                stats = p3.tile([128, 8, 12], F32)

                for lt in range(8):
                    qt0 = p3ps.tile([128, 512], F32, tag="qkvps", bufs=6)
                    qt1 = p3ps.tile([128, 512], F32, tag="qkvps", bufs=6)
                    vtp = p3ps.tile([128, 512], F32, tag="qkvps", bufs=6)
                    for t in range(CT):
                        nc.tensor.matmul(
                            qt0[:], xm[:, t, lt * 128:(lt + 1) * 128],
                            wet_sb[:, t, 0:512],
                            start=(t == 0), stop=False)
                    for t in range(CT):
                        nc.tensor.matmul(
                            qt0[:], xpos[:, t, lt * 128:(lt + 1) * 128],
                            wot_sb[:, t, 0:512],
                            start=False, stop=(t == CT - 1))
                    for t in range(CT):
                        nc.tensor.matmul(
                            qt1[:], xm[:, t, lt * 128:(lt + 1) * 128],
                            wet_sb[:, t, 512:1024],
                            start=(t == 0), stop=False)
                    for t in range(CT):
                        nc.tensor.matmul(
                            qt1[:], xpos[:, t, lt * 128:(lt + 1) * 128],
                            wot_sb[:, t, 512:1024],
                            start=False, stop=(t == CT - 1))
                    for t in range(CT):
                        nc.tensor.matmul(
                            vtp[:], xm[:, t, lt * 128:(lt + 1) * 128],
                            wvt_sb[:, t, :],
                            start=(t == 0), stop=(t == CT - 1))
                    sq3 = p3.tile([128, 1536], F32, tag="sq3", bufs=2)
                    nc.scalar.square(sq3[:, 0:512], qt0[:])
                    nc.scalar.square(sq3[:, 512:1024], qt1[:])
                    nc.scalar.square(sq3[:, 1024:1536], vtp[:])
                    nc.vector.tensor_reduce(
                        stats[:, lt, :],
                        sq3[:].rearrange("p (s d) -> p s d", d=128),
                        axis=mybir.AxisListType.X,
                        op=mybir.AluOpType.add)
                    inv_t = p3.tile([128, 12], F32, tag="inv12", bufs=2)
                    scr_t = p3.tile([128, 12], F32, tag="scr12", bufs=2)
                    tmp_t = p3.tile([128, 12], F32, tag="tmp12", bufs=2)
                    nc.scalar.activation(
                        tmp_t[:], stats[:, lt, :], AF.Sqrt, scale=1.0 / 128.0)
                    nc.vector.tensor_scalar_add(tmp_t[:], tmp_t[:], EPS)
                    nc.vector.reciprocal_approx_accurate(scr_t[:], tmp_t[:], inv_t[:])
                    # scr_t now holds 1/(eps+sqrt(ns/128)); apply
                    nc.vector.tensor_mul(
                        qkn[:, lt, 0:512],
                        qt0[:].rearrange("p (s d) -> p s d", d=128),
                        scr_t[:, 0:4, None].broadcast_to([128, 4, 128]))
                    nc.vector.tensor_mul(
                        qkn[:, lt, 512:1024],
                        qt1[:].rearrange("p (s d) -> p s d", d=128),
                        scr_t[:, 4:8, None].broadcast_to([128, 4, 128]))
                    nc.vector.tensor_mul(
                        vn[:, lt, :],
                        vtp[:].rearrange("p (s d) -> p s d", d=128),
                        scr_t[:, 8:12, None].broadcast_to([128, 4, 128]))

                # k transposes (fp16 payload) + v ship, then one merged AG;
                # q transposes overlap the collective
                agk_view = agm_in[0, 0:AGK].rearrange("(c l) -> c l", c=C)
                for h in range(HEADS):
                    for bank in range(2):
                        tp = p3ps.tile([128, 512], F16, tag="tps", bufs=2)
                        for i in range(4):
                            lt = bank * 4 + i
                            nc.tensor.transpose(
                                tp[:, i * 128:(i + 1) * 128],
                                qkn[:, lt, (2 * h + 1) * 128:(2 * h + 2) * 128],
                                identh[:])
                        kst = p3.tile([128, 512], F16, tag="kst", bufs=2)
                        nc.vector.tensor_copy(kst[:], tp[:])
                        nc.sync.dma_start(
                            agk_view[h * 128:(h + 1) * 128,
                                     bank * 512:(bank + 1) * 512],
                            kst[:])
                nc.sync.dma_start(
                    agm_in[0, AGK:].bitcast(F8)
                    .rearrange("(lt p d) -> p lt d", p=128, d=C), vn[:])
                nc.vector.tensor_scalar_mul(xh[:], xm[:], 0.5)
                if not skip_cc:
                    nc.gpsimd.collective_compute(
                        "AllGather", mybir.AluOpType.bypass,
                        replica_groups=[list(range(R))],
                        ins=[agm_in[:]], outs=[agm_out[:]])
                for h in range(HEADS):
                    for bank in range(2):
                        tp = p3ps.tile([128, 512], F16, tag="tps", bufs=2)
                        for i in range(4):
                            lt = bank * 4 + i
                            nc.tensor.transpose(
                                tp[:, i * 128:(i + 1) * 128],
                                qkn[:, lt, (2 * h) * 128:(2 * h + 1) * 128],
                                identh[:])
                        nc.vector.tensor_copy(
                            q_sb[:, h, bank * 512:(bank + 1) * 512], tp[:])

            # ============ Phase 4: attention ============
            # e = exp(s - SHIFT) stored fp8e4 (max e^5.31=202 < 240); the
            # SHIFT factor cancels between PV numerator and the denominator.
            # exp is split ACT (exact, cols < XSPLIT) / DVE (Schraudolph
            # bit-trick to uint8==fp8e4 pattern, saturating at 0 on
            # underflow). PV and the denominator are fp8 DoubleRow matmuls
            # contracting 256 keys per instruction; no DVE sum tree.
            with tc.tile_pool(name="p4", bufs=1) as p4, \
                 tc.tile_pool(name="p4ps", bufs=1, space="PSUM") as p4ps:
                # one full-d v tile for all heads: contiguous gather, head
                # slices taken directly by the PV matmuls
                def load_ksb(hh):
                    t_ = p4.tile([128, L], F16, tag="ksb", bufs=2)
                    for r_ in range(R):
                        nc.sync.dma_start(
                            t_[:, r_ * Lc:(r_ + 1) * Lc],
                            agm_out[r_, 0:AGK]
                            .rearrange("(c l) -> c l", c=C)
                            [hh * 128:(hh + 1) * 128, :])
                    return t_

                ksb_next = load_ksb(0)
                vsb = p4.tile([128, KT, C], F8)
                for r_ in range(R):
                    nc.sync.dma_start(
                        vsb[:, r_ * 8:(r_ + 1) * 8, :],
                        agm_out[r_, AGK:].bitcast(F8)
                        .rearrange("(t p d) -> p t d", p=128, d=C))
                SA = ISQ_D * SCH_A
                SB = 56.0 - SHIFT * SCH_A + SCH_C
                for h in range(HEADS):
                    ksb = ksb_next
                    # prefetch next head's k before this head's output write
                    # enters the (in-order) DMA queue
                    if h + 1 < HEADS:
                        ksb_next = load_ksb(h + 1)
                    for cch in range(2):
                        q_c = q_sb[:, h, cch * 512:(cch + 1) * 512]
                        pvps = p4ps.tile([128, 512], F32, tag="pvps", bufs=2)
                        dps = p4ps.tile([1, 512], F32, tag="dps", bufs=1)
                        e_prev = None

                        def pv_den(pg, e_):
                            nc.tensor.matmul(
                                pvps[:],
                                vsb[:, 2 * pg:2 * pg + 2,
                                    h * 128:(h + 1) * 128],
                                e_[:].rearrange("p (t q) -> p t q", q=512),
                                start=(pg == 0), stop=(pg == KT // 2 - 1),
                                perf_mode=DR)
                            nc.tensor.matmul(
                                dps[0:1, :],
                                ones8[:, :, 0:1],
                                e_[:].rearrange("p (t q) -> p t q", q=512),
                                start=(pg == 0), stop=(pg == KT // 2 - 1),
                                perf_mode=DR)

                        for grp in range(KT // 2):
                            sps = p4ps.tile([128, 1024], F32, tag="sps", bufs=2)
                            for j in range(2):
                                t = grp * 2 + j
                                nc.tensor.matmul(
                                    sps[:, j * 512:(j + 1) * 512],
                                    ksb[:, t * 128:(t + 1) * 128],
                                    q_c,
                                    start=True, stop=True)
                            e = p4.tile([128, 1024], F8, tag="e", bufs=4)
                            nc.scalar.activation(
                                e[:, 0:XSPLIT], sps[:, 0:XSPLIT], AF.Exp,
                                scale=ISQ_D, bias=nshift[:, 0:1])
                            nc.vector.tensor_scalar(
                                e[:, XSPLIT:].bitcast(U8), sps[:, XSPLIT:],
                                SA, SB,
                                op0=mybir.AluOpType.mult,
                                op1=mybir.AluOpType.add)
                            # PV+den for the PREVIOUS group: its exp has had a
                            # full score-round to finish, so PE never stalls
                            if e_prev is not None:
                                pv_den(grp - 1, e_prev)
                            e_prev = e
                        pv_den(KT // 2 - 1, e_prev)
                        inv_s = p4.tile([1, 512], F32, tag="invs", bufs=2)
                        scr_s = p4.tile([1, 512], F32, tag="scrs", bufs=2)
                        nc.vector.reciprocal_approx_accurate(
                            inv_s[:], dps[:], scr_s[:])
                        inv_sr = p4.tile([1, 512], F32R, tag="invsr", bufs=2)
                        nc.vector.tensor_copy(inv_sr[:], inv_s[:])
                        repl = p4ps.tile([128, 512], F32, tag="repl", bufs=1)
                        nc.tensor.matmul(
                            repl[:], halfr[:], inv_sr[0:1, :],
                            start=True, stop=True)
                        pv_sb = p4.tile([128, 512], F32, tag="pvsb", bufs=2)
                        nc.vector.tensor_copy(pv_sb[:], pvps[:])
                        nc.vector.tensor_mul(
                            xout[:, h, cch * 512:(cch + 1) * 512],
                            pv_sb[:],
                            repl[:])

                    # per-head final combine: out = clip(xm/2 + attn, +-256)
                    nc.vector.tensor_add(
                        xh[:, h, :], xh[:, h, :], xout[:, h, :])
                    nc.vector.tensor_scalar(
                        xh[:, h, :], xh[:, h, :], CLIP, -CLIP,
                        op0=mybir.AluOpType.min, op1=mybir.AluOpType.max)
                    nc.sync.dma_start(
                        out_d[:].rearrange("p (t l) -> p t l", l=Lc)[:, h, :],
                        xh[:, h, :])



def _host_prep(x, emb, pos_emb, emb_gain, w_res0, w_depth, w_emb, w_res1,
               w_qk, w_v):
    """Build shared weight arrays + per-core input shards."""
    f = np.float32
    w_res0 = np.asarray(w_res0, f).reshape(M, C)
    w_depth = np.asarray(w_depth, f).reshape(M, 128, 9)
    w_emb = np.asarray(w_emb, f).reshape(M, C)
    w_res1 = np.asarray(w_res1, f).reshape(C, M)
    w_qk = np.asarray(w_qk, f).reshape(2 * C, 2 * C)
    w_v = np.asarray(w_v, f).reshape(C, C)
    emb_gain = np.float32(emb_gain)

    w0t = np.ascontiguousarray((w_res0 * (1.0 / np.sqrt(C))).T)     # [C, M]
    wd = w_depth * (1.0 / np.sqrt(128 * 9))
    # wdt[p, (g*9+t)*128 + co] = wd[g*128+co, p, t]
    wdt = np.empty((128, 72 * 128), f)
    for g in range(MT):
        blk = wd[g * 128:(g + 1) * 128]          # [co=128, ci=128, tap=9]
        wdt[:, g * 9 * 128:(g + 1) * 9 * 128] = (
            blk.transpose(1, 2, 0).reshape(128, 9 * 128))
    wembt = np.ascontiguousarray((w_emb * (emb_gain / np.sqrt(C))).T)  # [C, M]
    w1t = np.ascontiguousarray(
        (w_res1 * (1.0 / (SILU_SCALE * np.sqrt(M)))).T)             # [M, C]

    # qk permutation: new row n = (2h+s)*128+dd  <- old row h*256 + dd*2 + s
    perm = np.empty(2 * C, np.int64)
    for h in range(HEADS):
        for s in range(2):
            for dd in range(128):
                perm[(2 * h + s) * 128 + dd] = h * 256 + dd * 2 + s
    wqk_p = w_qk[perm] * (1.0 / np.sqrt(2 * C))
    wet = np.ascontiguousarray((wqk_p[:, 0::2] * ISQ2).T)           # [C, M]
    wot = np.ascontiguousarray(wqk_p[:, 1::2].T)                    # [C, M]
    wvt = np.ascontiguousarray((w_v * (1.0 / np.sqrt(C) * ISQ2)).T)  # [C, C]

    x = np.asarray(x, f).reshape(C, 16, 512)
    pos = np.asarray(pos_emb, f).reshape(C, 16, 512) * ISQ2
    embv = np.ascontiguousarray(np.asarray(emb, f).reshape(C))

    def pmajor(a, nt):
        # [nt*128, F] -> [128, nt*F]: partition-major so the DMA streams
        # contiguously per partition
        fdim = a.shape[1]
        return np.ascontiguousarray(
            a.reshape(nt, 128, fdim).transpose(1, 0, 2).reshape(128, nt * fdim))

    shared = dict(
        embv=pmajor(embv.reshape(C, 1), CT),
        w0t=pmajor(w0t, CT), wdt=wdt, wembt=pmajor(wembt, CT),
        w1t=pmajor(w1t, MT), wet=pmajor(wet, CT), wot=pmajor(wot, CT),
        wvt=pmajor(wvt, CT),
        onec=np.ones((128, 1), f), oner=np.ones((1, 128), f),
        halfr=np.full((1, 128), ISQ2, f),
        onecb=np.ones((128, 1), ml_dtypes.bfloat16))
    in_maps = []
    for r in range(R):
        m = dict(shared)
        m["xs"] = pmajor(np.ascontiguousarray(
            x[:, 2 * r:2 * r + 2, :].reshape(C, Lc)), CT)
        m["pos"] = pmajor(np.ascontiguousarray(
            pos[:, 2 * r:2 * r + 2, :].reshape(C, Lc)), CT)
        in_maps.append(m)
    return in_maps


def kernel(**inputs):
    if "nc" not in _CACHE:
        _CACHE["nc"] = _build_nc()
    nc = _CACHE["nc"]
    in_maps = _host_prep(
        inputs["x"], inputs["emb"], inputs["pos_emb"], inputs["emb_gain"],
        inputs["w_res0"], inputs["w_depth"], inputs["w_emb"],
        inputs["w_res1"], inputs["w_qk"], inputs["w_v"])
    res = run_bass_kernel_spmd(nc, in_maps, list(range(R)))
    out = np.empty((1, C, 16, 512), np.float32)
    for r in range(R):
        o = res.results[r]["out"].reshape(128, CT, Lc).transpose(1, 0, 2)
        out[0, :, 2 * r:2 * r + 2, :] = o.reshape(C, 2, 512)
    return out

